# revision 1
# baseline (speedup 1.0000x reference)
"""NeuralMemory (scatter_memory) Trainium2 Bass kernel, 8-core SPMD.

Strategy:
  Phase A (data-parallel over all B*T tokens, 528/core, zero-padded to 640):
    project k/v/alr, run the 2-layer ResLinear forward + manual backward in
    feature-major ("T") layout with fp32r matmuls, PE-transpose the four
    dW operands into token-major layout, compute per-core partial dW^T.
  AllReduce the partial dW^T (bf16) across the 8 cores; the AdamW-style
    first step reduces to w_new = w*(1-lr*wd) - lr*sign(g), computed
    identically on every core.
  Phase C (each core owns one (batch, 512-token output range)): recompute
    queries + retrieval over own tokens + 512-token halo (padded to 1024),
    sliding-window attention in bf16 with relative-position triangle masks
    and an additive key-validity bias, output projection, write own slice.
"""
import numpy as np
import concourse.bass as bass
import concourse.tile as tile
import concourse.mybir as mybir
from concourse import bass_utils
import bass_rust

F32 = mybir.dt.float32
BF16 = mybir.dt.bfloat16
F32R = mybir.dt.float32r
AF = mybir.ActivationFunctionType
OP = mybir.AluOpType

NCORES = 8
B, S, D = 2, 2048, 512
M, C, H, WIN = 64, 16, 8, 512
N_LAYERS = 2
MAX_ALR = 0.01
LR, WD, EPS = 1e-3, 1e-2, 1e-8
T = M + S                  # 2112
NTOK = B * T               # 4224
TA = NTOK // NCORES        # 528 tokens/core in phase A
TAP = 640                  # padded phase-A width (5 x 128)
TC = 1024                  # phase-C halo+own width (8 x 128)
DT = D // 128              # 4 feature tiles
HD = D // H                # 64


def split_waits(nc):
    """This walrus build encodes at most ONE sync wait per instruction.
    Hoist excess waits onto injected EventSemaphore instructions."""
    n = 0
    for fn in nc.m.functions:
        for blk in fn.blocks:
            newl = []
            for ins in blk.instructions:
                si = ins.sync_info
                if si is not None and len(si.on_wait) > 1:
                    waits = list(si.on_wait)
                    for w in waits[:-1]:
                        ev = mybir.InstEventSemaphore(
                            name=f"{ins.name}_w{n}", ins=[], outs=[])
                        ev.engine = ins.engine
                        ev.sync_info = bass_rust.SyncInfo(on_wait=[w], on_update=[])
                        newl.append(ev)
                        n += 1
                    ins.sync_info = bass_rust.SyncInfo(
                        on_wait=[waits[-1]], on_update=list(si.on_update))
                newl.append(ins)
            blk.instructions[:] = newl
    return n


_UID = [0]


def blocks(pool, nblk, width, dtype, tag):
    _UID[0] += 1
    t = pool.tile([128, nblk, width], dtype, tag=tag, name=f"{tag}_u{_UID[0]}")
    return [t[:, i, :] for i in range(nblk)]


def build(nbody=1, sim=False):
    nc = bass.Bass("TRN2", target_bir_lowering=False, debug=False,
                   num_devices=1 if sim else NCORES)

    # ---- DRAM I/O ----
    xmT_a = nc.dram_tensor("xmT_a", [D, TAP], F32R, kind="ExternalInput").ap()
    xmT_c = nc.dram_tensor("xmT_c", [D, TC], F32R, kind="ExternalInput").ap()
    validk = nc.dram_tensor("validk", [TC], F32, kind="ExternalInput").ap()
    lmask = nc.dram_tensor("lmask", [128, 128], BF16, kind="ExternalInput").ap()
    umask = nc.dram_tensor("umask", [128, 128], BF16, kind="ExternalInput").ap()
    ident = nc.dram_tensor("ident", [128, 128], F32R, kind="ExternalInput").ap()
    identb = nc.dram_tensor("identb", [128, 128], BF16, kind="ExternalInput").ap()
    wkT = nc.dram_tensor("wkT", [D, D], F32R, kind="ExternalInput").ap()
    wvT = nc.dram_tensor("wvT", [D, D], F32R, kind="ExternalInput").ap()
    wlrT = nc.dram_tensor("wlrT", [D, 1], F32R, kind="ExternalInput").ap()
    w0T = nc.dram_tensor("w0T", [D, D], F32R, kind="ExternalInput").ap()
    w0Tf = nc.dram_tensor("w0Tf", [D, D], F32, kind="ExternalInput").ap()
    w1T = nc.dram_tensor("w1T", [D, D], F32R, kind="ExternalInput").ap()
    w1Tf = nc.dram_tensor("w1Tf", [D, D], F32, kind="ExternalInput").ap()
    w1n = nc.dram_tensor("w1n", [D, D], F32R, kind="ExternalInput").ap()
    wqT = nc.dram_tensor("wqT", [D, D], F32R, kind="ExternalInput").ap()
    swqT = nc.dram_tensor("swqT", [D, D], BF16, kind="ExternalInput").ap()
    swkT = nc.dram_tensor("swkT", [D, D], BF16, kind="ExternalInput").ap()
    swvT = nc.dram_tensor("swvT", [D, D], BF16, kind="ExternalInput").ap()
    swoT = nc.dram_tensor("swoT", [D, D], BF16, kind="ExternalInput").ap()
    out_d = nc.dram_tensor("out", [D, 512], F32, kind="ExternalOutput").ap()

    with tile.TileContext(nc) as tc:
        with (
            tc.tile_pool(name="wpool", bufs=1) as wp,      # persistent
            tc.tile_pool(name="dramp", bufs=1, space="DRAM") as dramp,
        ):
            def load_w(pool, src, name, dtype, tag=None):
                bl = blocks(pool, DT, D, dtype, tag or name)
                for i in range(DT):
                    nc.sync.dma_start(out=bl[i], in_=src[128 * i:128 * (i + 1), :])
                return bl

            ident_r = wp.tile([128, 128], F32R, tag="ident_r", name="ident_r")
            nc.sync.dma_start(out=ident_r, in_=ident)
            ident_b = wp.tile([128, 128], BF16, tag="ident_b", name="ident_b")
            nc.sync.dma_start(out=ident_b, in_=identb)
            # w_new^T holder (f32r, phase-C stationary); list [l][j]
            wnT_t = wp.tile([128, N_LAYERS, DT, D], F32R, tag="wnT", name="wnT")
            wnT = [[wnT_t[:, l, j, :] for j in range(DT)]
                   for l in range(N_LAYERS)]

            def one_body(body_i):
                # ================= PHASE A =================
                with (
                    tc.tile_pool(name="apool", bufs=2) as ap,
                    tc.tile_pool(name="apers", bufs=1) as aps,
                    tc.tile_pool(name="psA", bufs=2, space="PSUM") as psA,
                    tc.tile_pool(name="psTr", bufs=2, space="PSUM") as psTr,
                    tc.tile_pool(name="psDw", bufs=2, space="PSUM") as psDw,
                ):
                    wkT_r = load_w(aps, wkT, "wkT_r", F32R)
                    wvT_r = load_w(aps, wvT, "wvT_r", F32R)
                    w0T_r = load_w(aps, w0T, "w0T_r", F32R)
                    w1T_r = load_w(aps, w1T, "w1T_r", F32R)
                    w1n_r = load_w(aps, w1n, "w1n_r", F32R)
                    wlrT_r = aps.tile([128, DT, 1], F32R, tag="wlrT_r", name="wlrT_r")
                    for i in range(DT):
                        nc.sync.dma_start(out=wlrT_r[:, i, :],
                                          in_=wlrT[128 * i:128 * (i + 1), :])

                    xa = blocks(aps, DT, TAP, F32R, "xa")
                    for i in range(DT):
                        nc.sync.dma_start(out=xa[i], in_=xmT_a[128 * i:128 * (i + 1), :])

                    # prefill wnT = W_l^T * (1 - LR*WD); finalized after AllReduce
                    c1 = 1.0 - LR * WD
                    for l, wsrc in enumerate((w0Tf, w1Tf)):
                        for i in range(DT):
                            wf = ap.tile([128, D], F32, tag="wf", name=f"wf{l}_{i}")
                            nc.sync.dma_start(out=wf, in_=wsrc[128 * i:128 * (i + 1), :])
                            nc.gpsimd.tensor_scalar_mul(wnT[l][i], wf, c1)

                    HALVES = ((0, 320), (320, 320))

                    def mmT(wtiles, rhs_tiles, name, evac):
                        for hf, (off, w) in enumerate(HALVES):
                            pss = []
                            for do in range(DT):
                                ps = psA.tile([128, 320], F32, tag="Amm",
                                              name=f"{name}_ps{do}_{hf}")
                                for ki in range(DT):
                                    nc.tensor.matmul(
                                        ps,
                                        wtiles[ki][:, 128 * do:128 * (do + 1)],
                                        rhs_tiles[ki][:, off:off + w],
                                        start=(ki == 0), stop=(ki == DT - 1))
                                pss.append(ps)
                            evac(off, w, pss)

                    # k / v projections
                    kT = blocks(aps, DT, TAP, F32R, "kT")
                    mmT(wkT_r, xa, "kproj",
                        lambda off, w, pss: [nc.scalar.copy(
                            kT[do][:, off:off + w], pss[do]) for do in range(DT)])
                    vT = blocks(aps, DT, TAP, BF16, "vT")
                    mmT(wvT_r, xa, "vproj",
                        lambda off, w, pss: [nc.scalar.copy(
                            vT[do][:, off:off + w], pss[do]) for do in range(DT)])

                    # alr: row [1, TAP] halves then DRAM round-trip to [128, 5]
                    srow = ap.tile([1, TAP], F32, tag="srow", name="srow")
                    for hf, (off, w) in enumerate(HALVES):
                        pa = psA.tile([1, 320], F32, tag="Amm", name=f"alr{hf}")
                        for ki in range(DT):
                            nc.tensor.matmul(pa, wlrT_r[:, ki, :],
                                             xa[ki][:, off:off + w],
                                             start=(ki == 0), stop=(ki == DT - 1))
                        nc.scalar.activation(srow[:, off:off + w], pa, AF.Sigmoid)
                    nc.vector.tensor_scalar_mul(srow, srow, 2.0 * MAX_ALR / D)
                    sband = dramp.tile([1, TAP], F32, tag="sband", name="sband")
                    nc.sync.dma_start(out=sband, in_=srow)
                    s_td_t = aps.tile([128, 5], F32, tag="s_td", name="s_td")
                    nc.sync.dma_start(
                        out=s_td_t,
                        in_=sband.opt().rearrange("a (c p) -> (a p) c", p=128))
                    s_td = [s_td_t[:, i:i + 1] for i in range(5)]

                    # z0; x1 = k + silu(z0); d0  (batched ACT functions)
                    x1T = blocks(aps, DT, TAP, F32R, "x1T")
                    d0T = blocks(aps, DT, TAP, BF16, "d0T")

                    def z0_evac(off, w, pss):
                        sils = []
                        for do in range(DT):
                            sil = ap.tile([128, 320], F32, tag="silA",
                                          name=f"sil0_{do}_{off}")
                            nc.scalar.activation(sil, pss[do], AF.Silu)
                            sils.append(sil)
                        for do in range(DT):
                            nc.scalar.activation(d0T[do][:, off:off + w],
                                                 pss[do], AF.Derivative_silu)
                        for do in range(DT):
                            nc.vector.tensor_tensor(
                                x1T[do][:, off:off + w],
                                kT[do][:, off:off + w], sils[do], OP.add)
                    mmT(w0T_r, kT, "z0", z0_evac)

                    # z1; dx2 = (x1+silu(z1)) - v; dz1 = dx2*d1
                    dz1T = blocks(aps, DT, TAP, F32R, "dz1T")
                    dx2T = blocks(aps, DT, TAP, BF16, "dx2T")

                    def z1_evac(off, w, pss):
                        sils = []
                        for do in range(DT):
                            sil = ap.tile([128, 320], F32, tag="silA",
                                          name=f"sil1_{do}_{off}")
                            nc.scalar.activation(sil, pss[do], AF.Silu)
                            sils.append(sil)
                        d1s = []
                        for do in range(DT):
                            d1 = ap.tile([128, 320], F32, tag="d1A",
                                         name=f"d1_{do}_{off}")
                            nc.scalar.activation(d1, pss[do], AF.Derivative_silu)
                            d1s.append(d1)
                        for do in range(DT):
                            x2 = ap.tile([128, 320], F32, tag="x2A",
                                         name=f"x2_{do}_{off}")
                            nc.vector.tensor_tensor(x2, x1T[do][:, off:off + w],
                                                    sils[do], OP.add)
                            nc.vector.tensor_tensor(dx2T[do][:, off:off + w],
                                                    x2, vT[do][:, off:off + w],
                                                    OP.subtract)
                            nc.vector.tensor_tensor(dz1T[do][:, off:off + w],
                                                    dx2T[do][:, off:off + w],
                                                    d1s[do], OP.mult)
                    mmT(w1T_r, x1T, "z1", z1_evac)

                    # u = (dz1 @ W1)^T; dx1 = dx2 + u; dz0 = dx1*d0
                    dz0T = blocks(aps, DT, TAP, BF16, "dz0T")

                    def u_evac(off, w, pss):
                        for do in range(DT):
                            dx1 = ap.tile([128, 320], F32R, tag="dx1A",
                                          name=f"dx1_{do}_{off}")
                            nc.vector.tensor_tensor(dx1, dx2T[do][:, off:off + w],
                                                    pss[do], OP.add)
                            nc.vector.tensor_tensor(dz0T[do][:, off:off + w],
                                                    dx1, d0T[do][:, off:off + w],
                                                    OP.mult)
                    mmT(w1n_r, dz1T, "u", u_evac)

                    # ---- PE transposes into token-major [t, d] ----
                    k_td = blocks(aps, 5, D, F32R, "k_td")
                    x1_td = blocks(aps, 5, D, F32R, "x1_td")
                    sdz1_td = blocks(aps, 5, D, F32R, "sdz1_td")
                    sdz0_td = blocks(aps, 5, D, F32R, "sdz0_td")

                    def transpose_into(dst, src, scale_s, name):
                        bf = (src[0].dtype == BF16)
                        for tt in range(5):
                            for do in range(DT):
                                pt = psTr.tile([128, 128], BF16 if bf else F32R,
                                               tag="Atr", name=f"tr_{name}_{tt}_{do}")
                                nc.tensor.transpose(
                                    pt, src[do][:, 128 * tt:128 * (tt + 1)],
                                    ident_b if bf else ident_r)
                                dsl = dst[tt][:, 128 * do:128 * (do + 1)]
                                if scale_s:
                                    nc.vector.tensor_scalar(
                                        dsl, pt, s_td[tt], None, OP.mult)
                                elif do % 2 == 0:
                                    nc.scalar.copy(dsl, pt)
                                else:
                                    nc.vector.tensor_copy(dsl, pt)

                    transpose_into(k_td, kT, False, "k")
                    transpose_into(x1_td, x1T, False, "x1")
                    transpose_into(sdz1_td, dz1T, True, "dz1")
                    transpose_into(sdz0_td, dz0T, True, "dz0")

                    # ---- dW^T partials (bf16) + AllReduce + update ----
                    g_dram = dramp.tile([128, N_LAYERS * DT * D], BF16,
                                        tag="g_dram", name="g_dram")
                    gs_dram = dramp.tile([128, N_LAYERS * DT * D], BF16,
                                         tag="gs_dram", name="gs_dram")
                    for l, (x_td, z_td) in enumerate(((k_td, sdz0_td),
                                                      (x1_td, sdz1_td))):
                        for j in range(DT):
                            pdw = psDw.tile([128, D], F32, tag="Adw",
                                            name=f"dw_ps{l}_{j}")
                            for tt in range(5):
                                nc.tensor.matmul(
                                    pdw, x_td[tt][:, 128 * j:128 * (j + 1)],
                                    z_td[tt], start=(tt == 0), stop=(tt == 4))
                            gsb = ap.tile([128, D], BF16, tag="gsb",
                                          name=f"gsb{l}_{j}")
                            nc.vector.tensor_copy(gsb, pdw)
                            nc.sync.dma_start(
                                out=g_dram[:, (l * DT + j) * D:(l * DT + j + 1) * D],
                                in_=gsb)

                    if sim:
                        nc.gpsimd.dma_start(out=gs_dram, in_=g_dram)
                    else:
                        nc.gpsimd.collective_compute(
                            "AllReduce", OP.add,
                            replica_groups=[list(range(NCORES))],
                            ins=[g_dram.opt()], outs=[gs_dram.opt()])
                    for l in range(N_LAYERS):
                        for j in range(DT):
                            gsum = ap.tile([128, D], BF16, tag="gsum",
                                           name=f"gsum{l}_{j}")
                            nc.sync.dma_start(
                                out=gsum,
                                in_=gs_dram[:, (l * DT + j) * D:(l * DT + j + 1) * D])
                            sgn = ap.tile([128, D], F32, tag="sgn", name=f"sgn{l}_{j}")
                            nc.scalar.activation(sgn, gsum, AF.Sign)
                            nc.vector.scalar_tensor_tensor(
                                wnT[l][j], sgn, -LR, wnT[l][j], OP.mult, OP.add)

                # ================= PHASE C =================
                with (
                    tc.tile_pool(name="cpool", bufs=2) as cp,
                    tc.tile_pool(name="cpers", bufs=1) as cps,
                    tc.tile_pool(name="psC", bufs=3, space="PSUM") as psC,
                    tc.tile_pool(name="psS", bufs=3, space="PSUM") as psS,
                    tc.tile_pool(name="psAv", bufs=2, space="PSUM") as psAv,
                ):
                    wqT_r = load_w(cps, wqT, "wqT_r", F32R)
                    swqT_r = load_w(cps, swqT, "swqT_r", BF16)
                    swkT_r = load_w(cps, swkT, "swkT_r", BF16)
                    swvT_r = load_w(cps, swvT, "swvT_r", BF16)
                    swoT_b = load_w(cps, swoT, "swoT_b", BF16)
                    lmask_b = cps.tile([128, 128], BF16, tag="lmask_b", name="lmask_b")
                    nc.sync.dma_start(out=lmask_b, in_=lmask)
                    umask_b = cps.tile([128, 128], BF16, tag="umask_b", name="umask_b")
                    nc.sync.dma_start(out=umask_b, in_=umask)
                    vald = cps.tile([128, 8], F32, tag="vald", name="vald")
                    nc.sync.dma_start(out=vald,
                                      in_=validk.rearrange("(c p) -> p c", p=128))
                    xc = blocks(cps, DT, TC, F32R, "xc")
                    for i in range(DT):
                        nc.sync.dma_start(out=xc[i], in_=xmT_c[128 * i:128 * (i + 1), :])

                    def mmC(wtiles, rhs_tiles, name, out_cb, width=TC, roff=0):
                        for do in range(DT):
                            for off in range(0, width, 512):
                                ps = psC.tile([128, 512], F32, tag="Cmm",
                                              name=f"{name}_ps{do}_{off}")
                                for ki in range(DT):
                                    nc.tensor.matmul(
                                        ps, wtiles[ki][:, 128 * do:128 * (do + 1)],
                                        rhs_tiles[ki][:, roff + off:roff + off + 512],
                                        start=(ki == 0), stop=(ki == DT - 1))
                                out_cb(do, off, ps)

                    qT = blocks(cps, DT, TC, F32R, "qT")
                    mmC(wqT_r, xc, "q",
                        lambda do, off, ps: nc.scalar.copy(qT[do][:, off:off + 512], ps))

                    r0T = blocks(cps, DT, TC, F32R, "r0T")

                    def l0_out(do, off, ps):
                        sil = cp.tile([128, 512], F32, tag="silC", name=f"l0s{do}_{off}")
                        nc.scalar.activation(sil, ps, AF.Silu)
                        nc.vector.tensor_tensor(r0T[do][:, off:off + 512],
                                                qT[do][:, off:off + 512], sil, OP.add)
                    mmC(wnT[0], qT, "l0", l0_out)

                    rT = blocks(cps, DT, TC, BF16, "rT")

                    def l1_out(do, off, ps):
                        sil = cp.tile([128, 512], F32, tag="silC", name=f"l1s{do}_{off}")
                        nc.scalar.activation(sil, ps, AF.Silu)
                        nc.vector.tensor_tensor(rT[do][:, off:off + 512],
                                                r0T[do][:, off:off + 512], sil, OP.add)
                    mmC(wnT[1], r0T, "l1", l1_out)

                    kTb = blocks(cps, DT, TC, BF16, "kTb")
                    mmC(swkT_r, rT, "sk",
                        lambda do, off, ps: nc.scalar.copy(kTb[do][:, off:off + 512], ps))
                    qTb = blocks(cps, DT, 512, BF16, "qTb")
                    mmC(swqT_r, rT, "sq",
                        lambda do, off, ps: nc.scalar.copy(qTb[do], ps),
                        width=512, roff=512)

                    # v token-major with interleaved ones column: per kt [128, 8*65]
                    v65 = blocks(cps, 8, H * 65, BF16, "v65")
                    for kt in range(8):
                        pv = psC.tile([128, 512], F32, tag="Cmm", name=f"v_ps{kt}")
                        for ki in range(DT):
                            nc.tensor.matmul(pv, rT[ki][:, 128 * kt:128 * (kt + 1)],
                                             swvT_r[ki], start=(ki == 0),
                                             stop=(ki == DT - 1))
                        v3 = v65[kt].rearrange("p (h c) -> p h c", c=65)
                        nc.vector.tensor_copy(v3[:, :, 0:64],
                                              pv.rearrange("p (h c) -> p h c", c=64))
                        nc.vector.memset(v3[:, :, 64:65], 1.0)

                    # attention per head
                    oTb = blocks(cps, DT, 512, BF16, "oTb")
                    for h in range(H):
                        th, base = h // 2, 64 * (h % 2)
                        av = psAv.tile([65, 512], F32, tag="Av", name=f"av{h}")
                        dband = dramp.tile([1, 512], F32, tag="dband", name=f"db{h}")
                        for kt in range(8):
                            qlo = 128 * max(0, kt - 4)
                            qhi = min(512, 128 * (kt + 1))
                            wdt = qhi - qlo
                            sc = psS.tile([128, 512], F32, tag="Sc", name=f"sc{h}_{kt}")
                            nc.tensor.matmul(
                                sc[:, 0:wdt],
                                kTb[th][base:base + 64, 128 * kt:128 * (kt + 1)],
                                qTb[th][base:base + 64, qlo:qhi],
                                start=True, stop=True, tile_position=(base, 0))
                            pbf = cp.tile([128, 512], BF16, tag="Pbf",
                                          name=f"p{h}_{kt}")
                            nc.scalar.activation(pbf[:, 0:wdt], sc[:, 0:wdt], AF.Exp,
                                                 scale=0.125, bias=vald[:, kt:kt + 1])
                            if kt <= 3:
                                nc.vector.tensor_tensor(
                                    pbf[:, wdt - 128:wdt], pbf[:, wdt - 128:wdt],
                                    lmask_b, OP.mult)
                            if kt >= 4:
                                nc.vector.tensor_tensor(
                                    pbf[:, 0:128], pbf[:, 0:128], umask_b, OP.mult)
                            nc.tensor.matmul(
                                av[:, qlo:qhi], v65[kt][:, 65 * h:65 * h + 65],
                                pbf[:, 0:wdt], start=(kt == 0), stop=(kt == 7))
                        rden = cp.tile([1, 512], F32, tag="rden", name=f"rd{h}")
                        nc.vector.reciprocal(rden, av[64:65, :])
                        nc.sync.dma_start(out=dband, in_=rden)
                        rbc = cp.tile([64, 512], F32, tag="rbc", name=f"rbc{h}")
                        nc.gpsimd.dma_start(out=rbc,
                                            in_=dband.opt().partition_broadcast(64))
                        nc.vector.tensor_tensor(oTb[th][base:base + 64, :],
                                                av[0:64, :], rbc, OP.mult)

                    # output projection + store
                    for do in range(DT):
                        po = psC.tile([128, 512], F32, tag="Cmm", name=f"o_ps{do}")
                        for ki in range(DT):
                            nc.tensor.matmul(po, swoT_b[ki][:, 128 * do:128 * (do + 1)],
                                             oTb[ki], start=(ki == 0),
                                             stop=(ki == DT - 1))
                        ofin = cp.tile([128, 512], F32, tag="ofin", name=f"ofin{do}")
                        nc.scalar.copy(ofin, po)
                        nc.sync.dma_start(out=out_d[128 * do:128 * (do + 1), :],
                                          in_=ofin)

            for _bi in range(nbody):
                one_body(_bi)
    return nc


_CACHE = {}


def _get_nc(nbody=1):
    key = f"nc{nbody}"
    if key not in _CACHE:
        nc = build(nbody)
        split_waits(nc)
        _CACHE[key] = nc
    return _CACHE[key]


def prepare_in_maps(x, meta_memory, lmm_w, w_q, w_k, w_v, w_lr,
                    swa_wq, swa_wk, swa_wv, swa_wo):
    x = np.asarray(x, np.float32)
    meta_memory = np.asarray(meta_memory, np.float32)
    lmm_w = np.asarray(lmm_w, np.float32)
    xm = np.concatenate(
        [np.broadcast_to(meta_memory, (B,) + meta_memory.shape), x], axis=1)
    xf = xm.reshape(NTOK, D)

    import ml_dtypes
    bfd = ml_dtypes.bfloat16
    tri = np.arange(128)
    lmask_np = (tri[None, :] < tri[:, None]).astype(bfd)   # qj < ki
    umask_np = (tri[None, :] >= tri[:, None]).astype(bfd)  # qj >= ki
    ident_np = np.eye(128, dtype=np.float32)

    common = {
        "lmask": lmask_np, "umask": umask_np, "ident": ident_np,
        "identb": ident_np.astype(bfd),
        "wkT": np.ascontiguousarray(np.asarray(w_k, np.float32).T),
        "wvT": np.ascontiguousarray(np.asarray(w_v, np.float32).T),
        "wlrT": np.ascontiguousarray(np.asarray(w_lr, np.float32).T),
        "w0T": np.ascontiguousarray(lmm_w[0].T),
        "w0Tf": np.ascontiguousarray(lmm_w[0].T),
        "w1T": np.ascontiguousarray(lmm_w[1].T),
        "w1Tf": np.ascontiguousarray(lmm_w[1].T),
        "w1n": np.ascontiguousarray(lmm_w[1]),
        "wqT": np.ascontiguousarray(np.asarray(w_q, np.float32).T),
        "swqT": np.ascontiguousarray(np.asarray(swa_wq, np.float32).T).astype(bfd),
        "swkT": np.ascontiguousarray(np.asarray(swa_wk, np.float32).T).astype(bfd),
        "swvT": np.ascontiguousarray(np.asarray(swa_wv, np.float32).T).astype(bfd),
        "swoT": np.ascontiguousarray(np.asarray(swa_wo, np.float32).T).astype(bfd),
    }
    in_maps = []
    for c in range(NCORES):
        xa = np.zeros((D, TAP), np.float32)
        xa[:, :TA] = xf[TA * c:TA * (c + 1)].T
        b, r = c // 4, c % 4
        t1 = M + 512 * (r + 1)
        lo = max(t1 - TC, 0)
        pad = TC - (t1 - lo)
        xcm = np.zeros((D, TC), np.float32)
        xcm[:, pad:] = xm[b, lo:t1].T
        vk = np.full(TC, -30.0, np.float32)
        vk[pad:] = 0.0
        mcore = dict(common)
        mcore["xmT_a"] = xa
        mcore["xmT_c"] = xcm
        mcore["validk"] = vk
        in_maps.append(mcore)
    return in_maps


def run_on_device(in_maps, nbody=1):
    nc = _get_nc(nbody)
    return bass_utils.run_bass_kernel_spmd(nc, in_maps,
                                           core_ids=list(range(NCORES)))


def kernel(**inputs):
    in_maps = prepare_in_maps(**inputs)
    res = run_on_device(in_maps)
    out = np.empty((B, S, D), np.float32)
    for c in range(NCORES):
        b, r = c // 4, c % 4
        out[b, 512 * r:512 * (r + 1), :] = res.results[c]["out"].T
    return out



# revision 2
# speedup vs baseline: 7913.5547x; 7913.5547x over previous
"""NeuralMemory (scatter_memory) Trainium2 Bass kernel, 8-core SPMD. v2

Strategy (per core):
  Phase A (data-parallel over all B*T tokens, 528/core, zero-padded to 640):
    bf16 feature-major projections k/v/alr, 2-layer ResLinear forward +
    manual backward, PE-transpose the four dW operands into token-major
    layout (batched 4-wide PSUM evacuations), per-core partial dW^T into
    one fused [128, 4096] bf16 buffer.
  AllReduce the partial dW^T (bf16); the first AdamW step from zero state
    reduces to w_new = w*(1-lr*wd) - lr*sign(g).  Phase C weight/x loads
    and the q-projection overlap the collective.
  Phase C (each core owns one (batch, 512-token output range)): retrieval
    over own tokens + 512-token halo (padded to 1024) in f32r, sliding-
    window attention in bf16; key validity folded into the v-ones column;
    softmax denominators broadcast with K=1 PE matmuls (no DRAM round
    trips); output projection, write own slice.
"""
import numpy as np
import concourse.bass as bass
import concourse.tile as tile
import concourse.mybir as mybir
from concourse import bass_utils
import bass_rust

F32 = mybir.dt.float32
BF16 = mybir.dt.bfloat16
F32R = mybir.dt.float32r
AF = mybir.ActivationFunctionType
OP = mybir.AluOpType

NCORES = 8
B, S, D = 2, 2048, 512
M, C, H, WIN = 64, 16, 8, 512
N_LAYERS = 2
MAX_ALR = 0.01
LR, WD, EPS = 1e-3, 1e-2, 1e-8
T = M + S                  # 2112
NTOK = B * T               # 4224
TA = NTOK // NCORES        # 528 tokens/core in phase A
TAP = 640                  # padded phase-A width (5 x 128)
TC = 1024                  # phase-C halo+own width (8 x 128)
DT = D // 128              # 4 feature tiles
TT = TAP // 128            # 5 token tiles in phase A
HD = D // H                # 64
HALVES = ((0, 512), (512, 128))


def split_waits(nc):
    """This walrus build encodes at most ONE sync wait per instruction.
    Hoist excess waits onto injected EventSemaphore instructions."""
    n = 0
    for fn in nc.m.functions:
        for blk in fn.blocks:
            newl = []
            for ins in blk.instructions:
                si = ins.sync_info
                if si is not None and len(si.on_wait) > 1:
                    waits = list(si.on_wait)
                    for w in waits[:-1]:
                        ev = mybir.InstEventSemaphore(
                            name=f"{ins.name}_w{n}", ins=[], outs=[])
                        ev.engine = ins.engine
                        ev.sync_info = bass_rust.SyncInfo(on_wait=[w], on_update=[])
                        newl.append(ev)
                        n += 1
                    ins.sync_info = bass_rust.SyncInfo(
                        on_wait=[waits[-1]], on_update=list(si.on_update))
                newl.append(ins)
            blk.instructions[:] = newl
    return n


_UID = [0]


def blocks(pool, nblk, width, dtype, tag):
    _UID[0] += 1
    t = pool.tile([128, nblk, width], dtype, tag=tag, name=f"{tag}_u{_UID[0]}")
    return [t[:, i, :] for i in range(nblk)]


def build(nbody=1, sim=False):
    nc = bass.Bass("TRN2", target_bir_lowering=False, debug=False,
                   num_devices=1 if sim else NCORES)

    # ---- DRAM I/O (bf16 throughout except the f32 output) ----
    xaT = nc.dram_tensor("xaT", [D, TAP], BF16, kind="ExternalInput").ap()
    xcT = nc.dram_tensor("xcT", [D, TC], BF16, kind="ExternalInput").ap()
    vald01 = nc.dram_tensor("vald01", [128, 8], F32, kind="ExternalInput").ap()
    lmask = nc.dram_tensor("lmask", [128, 128], BF16, kind="ExternalInput").ap()
    umask = nc.dram_tensor("umask", [128, 128], BF16, kind="ExternalInput").ap()
    identb = nc.dram_tensor("identb", [128, 128], BF16, kind="ExternalInput").ap()
    wkT = nc.dram_tensor("wkT", [D, D], BF16, kind="ExternalInput").ap()
    wvT = nc.dram_tensor("wvT", [D, D], BF16, kind="ExternalInput").ap()
    wlrT = nc.dram_tensor("wlrT", [D, 1], BF16, kind="ExternalInput").ap()
    w0T = nc.dram_tensor("w0T", [D, D], BF16, kind="ExternalInput").ap()
    w1T = nc.dram_tensor("w1T", [D, D], BF16, kind="ExternalInput").ap()
    w1n = nc.dram_tensor("w1n", [D, D], BF16, kind="ExternalInput").ap()
    wqT = nc.dram_tensor("wqT", [D, D], BF16, kind="ExternalInput").ap()
    swqT = nc.dram_tensor("swqT", [D, D], BF16, kind="ExternalInput").ap()
    swkT = nc.dram_tensor("swkT", [D, D], BF16, kind="ExternalInput").ap()
    swvT = nc.dram_tensor("swvT", [D, D], BF16, kind="ExternalInput").ap()
    swoT = nc.dram_tensor("swoT", [D, D], BF16, kind="ExternalInput").ap()
    out_d = nc.dram_tensor("out", [D, 512], F32, kind="ExternalOutput").ap()

    GW = N_LAYERS * DT * D     # 4096: fused gradient width

    with tile.TileContext(nc) as tc:
        with (
            tc.tile_pool(name="wpool", bufs=1) as wp,      # persistent
            tc.tile_pool(name="dramp", bufs=1, space="DRAM") as dramp,
        ):
            def load_w(src, name, eng=None):
                _UID[0] += 1
                t = wp.tile([128, DT, D], BF16, tag=name,
                            name=f"{name}_u{_UID[0]}")
                (eng or nc.sync).dma_start(
                    out=t, in_=src.rearrange("(a p) d -> p a d", p=128))
                return t, [t[:, i, :] for i in range(DT)]

            def one_body(body_i):
                # ---- bulk loads ----
                # phase A on the SP queue, most-urgent first
                xa_t = wp.tile([128, DT, TAP], BF16, tag="xa",
                               name=f"xa{body_i}")
                nc.sync.dma_start(
                    out=xa_t, in_=xaT.rearrange("(a p) d -> p a d", p=128))
                xa = [xa_t[:, i, :] for i in range(DT)]
                wlrT_r = wp.tile([128, DT, 1], BF16, tag="wlrT_r",
                                 name=f"wlr{body_i}")
                nc.sync.dma_start(
                    out=wlrT_r, in_=wlrT.rearrange("(a p) d -> p a d", p=128))
                _, wkT_r = load_w(wkT, "wkT_r")
                _, wvT_r = load_w(wvT, "wvT_r")
                w0T_t, w0T_r = load_w(w0T, "w0T_r")
                w1T_t, w1T_r = load_w(w1T, "w1T_r")
                _, w1n_r = load_w(w1n, "w1n_r")
                ident_b = wp.tile([128, 128], BF16, tag="ident_b",
                                  name=f"identb{body_i}")
                nc.sync.dma_start(out=ident_b, in_=identb)
                # phase C loads via SWDGE (Pool queue), off the SP HWDGE path
                _, wqT_r = load_w(wqT, "wqT_r", nc.gpsimd)
                xc_t = wp.tile([128, DT, TC], BF16, tag="xc",
                               name=f"xc{body_i}")
                nc.gpsimd.dma_start(
                    out=xc_t, in_=xcT.rearrange("(a p) d -> p a d", p=128))
                xc = [xc_t[:, i, :] for i in range(DT)]
                _, swqT_r = load_w(swqT, "swqT_r", nc.gpsimd)
                _, swkT_r = load_w(swkT, "swkT_r", nc.gpsimd)
                _, swvT_r = load_w(swvT, "swvT_r", nc.gpsimd)
                _, swoT_b = load_w(swoT, "swoT_b", nc.gpsimd)
                lmask_b = wp.tile([128, 128], BF16, tag="lmask_b",
                                  name=f"lm{body_i}")
                nc.gpsimd.dma_start(out=lmask_b, in_=lmask)
                umask_b = wp.tile([128, 128], BF16, tag="umask_b",
                                  name=f"um{body_i}")
                nc.gpsimd.dma_start(out=umask_b, in_=umask)
                vald_b = wp.tile([128, 8], F32, tag="vald_b",
                                 name=f"vald{body_i}")
                nc.gpsimd.dma_start(out=vald_b, in_=vald01)

                # w_new^T holder: [128, l, ki, D] f32r; prefill c1 * W^T
                wnT_t = wp.tile([128, N_LAYERS, DT, D], F32R, tag="wnT",
                                name=f"wnT{body_i}")
                wnT = [[wnT_t[:, l, j, :] for j in range(DT)]
                       for l in range(N_LAYERS)]
                c1 = 1.0 - LR * WD
                for l, wsrc in enumerate((w0T_t, w1T_t)):
                    nc.gpsimd.tensor_scalar_mul(wnT_t[:, l, :, :], wsrc, c1)

                # ================= PHASE A =================
                with (
                    tc.tile_pool(name="apool", bufs=3) as ap,
                    tc.tile_pool(name="apers", bufs=1) as aps,
                    tc.tile_pool(name="psA", bufs=4, space="PSUM") as psA,
                    tc.tile_pool(name="psTr", bufs=2, space="PSUM") as psTr,
                    tc.tile_pool(name="psDw", bufs=2, space="PSUM") as psDw,
                ):
                    def mmT(wtiles, rhs_tiles, name, evac):
                        for do in range(DT):
                            pss = []
                            for hf, (off, w) in enumerate(HALVES):
                                ps = psA.tile([128, 512], F32, tag="Amm",
                                              name=f"{name}_ps{do}_{hf}")
                                pss.append(ps[:, 0:w])
                            for ki in range(DT):
                                for hf, (off, w) in enumerate(HALVES):
                                    nc.tensor.matmul(
                                        pss[hf],
                                        wtiles[ki][:, 128 * do:128 * (do + 1)],
                                        rhs_tiles[ki][:, off:off + w],
                                        start=(ki == 0), stop=(ki == DT - 1))
                            for hf, (off, w) in enumerate(HALVES):
                                evac(do, off, w, pss[hf])

                    # alr first (row [1,TAP] -> DRAM round trip -> [128,5])
                    srow = aps.tile([1, TAP], F32, tag="srow", name="srow")
                    pa = psA.tile([128, 512], F32, tag="Amm", name="alr_ps")
                    pa2 = psA.tile([128, 512], F32, tag="Amm", name="alr_ps2")
                    for hf, (off, w) in enumerate(HALVES):
                        pz = (pa, pa2)[hf]
                        for ki in range(DT):
                            nc.tensor.matmul(pz[0:1, 0:w], wlrT_r[:, ki, :],
                                             xa[ki][:, off:off + w],
                                             start=(ki == 0), stop=(ki == DT - 1))
                        nc.scalar.activation(srow[:, off:off + w],
                                             pz[0:1, 0:w], AF.Sigmoid)
                    nc.vector.tensor_scalar_mul(srow, srow, 2.0 * MAX_ALR / D)
                    sband = dramp.tile([1, TAP], F32, tag="sband", name="sband")
                    nc.sync.dma_start(out=sband, in_=srow)
                    s_td = aps.tile([128, TT], F32, tag="s_td", name="s_td")
                    nc.sync.dma_start(
                        out=s_td,
                        in_=sband.opt().rearrange("a (c p) -> (a p) c", p=128))

                    # k projection
                    kT = blocks(aps, DT, TAP, BF16, "kT")
                    mmT(wkT_r, xa, "kproj",
                        lambda do, off, w, ps: nc.scalar.copy(
                            kT[do][:, off:off + w], ps))

                    # v projection (vTn = -v) immediately after: fills the
                    # PE gap while the z0 stage waits on kT evacuations
                    vTn = blocks(aps, DT, TAP, BF16, "vTn")
                    mmT(wvT_r, xa, "vproj",
                        lambda do, off, w, ps: nc.scalar.activation(
                            vTn[do][:, off:off + w], ps, AF.Copy, scale=-1.0))

                    # z0; x1 = k + silu(z0); d0
                    x1T = blocks(aps, DT, TAP, BF16, "x1T")
                    d0T = blocks(aps, DT, TAP, BF16, "d0T")

                    def z0_evac(do, off, w, ps):
                        sil = ap.tile([128, 512], F32, tag="silA",
                                      name=f"sil0_{do}_{off}")
                        nc.scalar.activation(sil[:, 0:w], ps, AF.Silu)
                        nc.scalar.activation(d0T[do][:, off:off + w],
                                             ps, AF.Derivative_silu)
                        nc.vector.tensor_tensor(
                            x1T[do][:, off:off + w],
                            kT[do][:, off:off + w], sil[:, 0:w], OP.add)
                    mmT(w0T_r, kT, "z0", z0_evac)

                    # token-major transpose targets, interleaved with the
                    # matmul stages to keep the PE fed
                    k_td = blocks(aps, TT, D, BF16, "k_td")
                    x1_td = blocks(aps, TT, D, BF16, "x1_td")
                    sdz1_td = blocks(aps, TT, D, BF16, "sdz1_td")
                    sdz0_td = blocks(aps, TT, D, BF16, "sdz0_td")

                    def transpose_into(dst, src, scale_s, name, eng):
                        for tt in range(TT):
                            pt = psTr.tile([128, DT, 128], BF16, tag="Atr",
                                           name=f"tr_{name}_{tt}")
                            for do in range(DT):
                                nc.tensor.transpose(
                                    pt[:, do, :],
                                    src[do][:, 128 * tt:128 * (tt + 1)],
                                    ident_b)
                            if scale_s:
                                nc.vector.tensor_scalar(
                                    dst[tt], pt, s_td[:, tt:tt + 1], None,
                                    OP.mult)
                            else:
                                eng(dst[tt], pt)

                    gsb = aps.tile([128, GW], BF16, tag="gsb", name="gsb")

                    def dw_partial(l, x_td, z_td):
                        for j in range(DT):
                            pdw = psDw.tile([128, D], F32, tag="Adw",
                                            name=f"dw_ps{l}_{j}")
                            for tt in range(TT):
                                nc.tensor.matmul(
                                    pdw, x_td[tt][:, 128 * j:128 * (j + 1)],
                                    z_td[tt], start=(tt == 0),
                                    stop=(tt == TT - 1))
                            eng = (nc.scalar.copy if (l * DT + j) % 2 == 0
                                   else nc.vector.tensor_copy)
                            eng(gsb[:, (l * DT + j) * D:(l * DT + j + 1) * D],
                                pdw)

                    transpose_into(k_td, kT, False, "k", nc.scalar.copy)


                    # z1; dx2 = (x1+silu(z1)) - v; dz1 = dx2*d1
                    dz1T = blocks(aps, DT, TAP, BF16, "dz1T")
                    dx2T = blocks(aps, DT, TAP, BF16, "dx2T")

                    def z1_evac(do, off, w, ps):
                        sil = ap.tile([128, 512], F32, tag="silA",
                                      name=f"sil1_{do}_{off}")
                        nc.scalar.activation(sil[:, 0:w], ps, AF.Silu)
                        d1 = ap.tile([128, 512], F32, tag="d1A",
                                     name=f"d1_{do}_{off}")
                        nc.scalar.activation(d1[:, 0:w], ps, AF.Derivative_silu)
                        x2 = ap.tile([128, 512], F32, tag="x2A",
                                     name=f"x2_{do}_{off}")
                        nc.vector.tensor_tensor(x2[:, 0:w],
                                                x1T[do][:, off:off + w],
                                                sil[:, 0:w], OP.add)
                        nc.vector.tensor_tensor(dx2T[do][:, off:off + w],
                                                x2[:, 0:w],
                                                vTn[do][:, off:off + w], OP.add)
                        nc.gpsimd.tensor_tensor(dz1T[do][:, off:off + w],
                                                dx2T[do][:, off:off + w],
                                                d1[:, 0:w], OP.mult)
                    mmT(w1T_r, x1T, "z1", z1_evac)

                    transpose_into(x1_td, x1T, False, "x1",
                                   nc.vector.tensor_copy)

                    # u = (dz1 @ W1)^T; dx1 = dx2 + u; dz0 = dx1*d0
                    dz0T = blocks(aps, DT, TAP, BF16, "dz0T")

                    def u_evac(do, off, w, ps):
                        dx1 = ap.tile([128, 512], BF16, tag="dx1A",
                                      name=f"dx1_{do}_{off}")
                        nc.vector.tensor_tensor(dx1[:, 0:w],
                                                dx2T[do][:, off:off + w],
                                                ps, OP.add)
                        nc.vector.tensor_tensor(dz0T[do][:, off:off + w],
                                                dx1[:, 0:w],
                                                d0T[do][:, off:off + w], OP.mult)
                    mmT(w1n_r, dz1T, "u", u_evac)

                    # layer-1 partials first: their operands finish earliest
                    transpose_into(sdz1_td, dz1T, True, "dz1", None)
                    dw_partial(1, x1_td, sdz1_td)
                    transpose_into(sdz0_td, dz0T, True, "dz0", None)
                    dw_partial(0, k_td, sdz0_td)

                    g_dram = dramp.tile([128, GW], BF16, tag="g_dram",
                                        name="g_dram")
                    gs_dram = dramp.tile([128, GW], BF16, tag="gs_dram",
                                         name="gs_dram")
                    nc.sync.dma_start(out=g_dram, in_=gsb)

                    if sim:
                        nc.gpsimd.dma_start(out=gs_dram, in_=g_dram)
                    else:
                        nc.gpsimd.collective_compute(
                            "AllReduce", OP.add,
                            replica_groups=[list(range(NCORES))],
                            ins=[g_dram.opt()], outs=[gs_dram.opt()])

                # ================= PHASE C =================
                with (
                    tc.tile_pool(name="cpool", bufs=2) as cp,
                    tc.tile_pool(name="cpb", bufs=4) as cpb,
                    tc.tile_pool(name="cpers", bufs=1) as cps,
                    tc.tile_pool(name="psC", bufs=3, space="PSUM") as psC,
                    tc.tile_pool(name="psS", bufs=2, space="PSUM") as psS,
                    tc.tile_pool(name="psAv", bufs=3, space="PSUM") as psAv,
                ):
                    def mmC(wtiles, rhs_tiles, name, out_cb, width=TC, roff=0,
                            offs=None):
                        if offs is None:
                            offs = range(0, width, 512)
                        for off in offs:
                            for do in range(DT):
                                ps = psC.tile([128, 512], F32, tag="Cmm",
                                              name=f"{name}_ps{do}_{off}")
                                for ki in range(DT):
                                    nc.tensor.matmul(
                                        ps, wtiles[ki][:, 128 * do:128 * (do + 1)],
                                        rhs_tiles[ki][:, roff + off:roff + off + 512],
                                        start=(ki == 0), stop=(ki == DT - 1))
                                out_cb(do, off, ps)

                    # queries: overlap the AllReduce (no dependency on wnT)
                    qT = blocks(cps, DT, TC, F32R, "qT")
                    mmC(wqT_r, xc, "q",
                        lambda do, off, ps: nc.scalar.copy(qT[do][:, off:off + 512], ps))

                    # AllReduce result -> sign -> w_new, chunked per layer/ki
                    # so the l0 matmuls start after the first chunk lands
                    gsum = cps.tile([128, GW], BF16, tag="gsum", name="gsum")
                    for hh in range(2):
                        sl = slice(hh * GW // 2, (hh + 1) * GW // 2)
                        nc.sync.dma_start(out=gsum[:, sl], in_=gs_dram[:, sl])
                    sgn = cps.tile([128, GW], BF16, tag="sgn", name="sgn")
                    for hh in range(2):
                        for ki in range(DT):
                            sl = slice((hh * DT + ki) * D,
                                       (hh * DT + ki + 1) * D)
                            nc.scalar.activation(sgn[:, sl], gsum[:, sl],
                                                 AF.Sign)
                            nc.vector.scalar_tensor_tensor(
                                wnT_t[:, hh, ki, :], sgn[:, sl], -LR,
                                wnT_t[:, hh, ki, :], OP.mult, OP.add)

                    r0T = blocks(cps, DT, TC, F32R, "r0T")

                    def l0_out(do, off, ps):
                        sil = cp.tile([128, 512], F32, tag="silC", name=f"l0s{do}_{off}")
                        nc.scalar.activation(sil, ps, AF.Silu)
                        nc.vector.tensor_tensor(r0T[do][:, off:off + 512],
                                                qT[do][:, off:off + 512], sil, OP.add)
                    mmC(wnT[0], qT, "l0", l0_out)

                    rT = blocks(cps, DT, TC, BF16, "rT")

                    def l1_out(do, off, ps):
                        sil = cp.tile([128, 512], F32, tag="silC", name=f"l1s{do}_{off}")
                        nc.scalar.activation(sil, ps, AF.Silu)
                        nc.vector.tensor_tensor(rT[do][:, off:off + 512],
                                                r0T[do][:, off:off + 512], sil, OP.add)
                    # own-token half (512:) first: it feeds sq, which gates
                    # every attention score matmul
                    mmC(wnT[1], r0T, "l1", l1_out, offs=(512, 0))

                    qTb = blocks(cps, DT, 512, BF16, "qTb")
                    mmC(swqT_r, rT, "sq",
                        lambda do, off, ps: nc.vector.tensor_copy(qTb[do], ps),
                        width=512, roff=512)

                    kTb = blocks(cps, DT, TC, BF16, "kTb")
                    v65 = blocks(cps, 8, H * 65, BF16, "v65")
                    ones8 = cps.tile([128, H, 1], BF16, tag="ones8", name="ones8")
                    nc.vector.memset(ones8, 1.0)

                    def v65_for(kt):
                        pv = psC.tile([128, 512], F32, tag="Cmm", name=f"v_ps{kt}")
                        for ki in range(DT):
                            nc.tensor.matmul(pv, rT[ki][:, 128 * kt:128 * (kt + 1)],
                                             swvT_r[ki], start=(ki == 0),
                                             stop=(ki == DT - 1))
                        v3 = v65[kt].rearrange("p (h c) -> p h c", c=65)
                        nc.vector.tensor_copy(v3[:, :, 0:64],
                                              pv.rearrange("p (h c) -> p h c", c=64))
                        nc.vector.tensor_scalar(v3[:, :, 64:65], ones8,
                                                vald_b[:, kt:kt + 1], None,
                                                OP.mult)

                    mmC(swkT_r, rT, "sk",
                        lambda do, off, ps: nc.vector.tensor_copy(
                            kTb[do][:, off:off + 512], ps),
                        offs=(512,))
                    for kt in range(4, 8):
                        v65_for(kt)
                    mmC(swkT_r, rT, "sk",
                        lambda do, off, ps: nc.vector.tensor_copy(
                            kTb[do][:, off:off + 512], ps),
                        offs=(0,))
                    for kt in range(4):
                        v65_for(kt)

                    # attention per head; denominators via K=1 PE broadcast.
                    # Head epilogues are software-pipelined one head behind so
                    # the PE never waits on the DVE reciprocal.
                    oTb = blocks(cps, DT, 512, BF16, "oTb")
                    rdena = cps.tile([1, H, 512], BF16, tag="rdena",
                                     name="rdena")
                    avs = [None] * H

                    onesb = cps.tile([1, 64], BF16, tag="onesb", name="onesb")
                    nc.vector.memset(onesb, 1.0)

                    def head_epilogue(h):
                        th, base = h // 2, 64 * (h % 2)
                        rbc = psC.tile([128, 512], F32, tag="Cmm",
                                       name=f"rbc{h}")
                        nc.tensor.matmul(rbc[0:64, :], onesb, rdena[:, h, :],
                                         start=True, stop=True)
                        rbs = cp.tile([64, 512], F32R, tag="rbs",
                                      name=f"rbs{h}")
                        nc.scalar.copy(rbs, rbc[0:64, :])
                        nc.vector.tensor_tensor(oTb[th][base:base + 64, :],
                                                avs[h][0:64, :], rbs,
                                                OP.mult)

                    # own-token key tiles (kt 4-7) are ready first, so they
                    # lead the PSUM accumulation order
                    KTQ = [(kt, 128 * max(0, kt - 4), min(512, 128 * (kt + 1)))
                           for kt in (4, 5, 6, 7, 0, 1, 2, 3)]

                    def emit_sc(h, kt, qlo, qhi):
                        # scores + exp + window masks -> pbf; returns pbf
                        th, base = h // 2, 64 * (h % 2)
                        wdt = qhi - qlo
                        sc = psS.tile([128, 512], F32, tag="Sc",
                                      name=f"sc{h}_{kt}")
                        nc.tensor.matmul(
                            sc[:, 0:wdt],
                            kTb[th][base:base + 64, 128 * kt:128 * (kt + 1)],
                            qTb[th][base:base + 64, qlo:qhi],
                            start=True, stop=True, tile_position=(base, 0))
                        pbf = cpb.tile([128, 512], BF16, tag="Pbf",
                                       name=f"p{h}_{kt}")
                        nc.scalar.activation(pbf[:, 0:wdt], sc[:, 0:wdt],
                                             AF.Exp, scale=0.125)
                        if kt <= 3:
                            nc.vector.tensor_tensor(
                                pbf[:, wdt - 128:wdt], pbf[:, wdt - 128:wdt],
                                lmask_b, OP.mult)
                        if kt >= 4:
                            nc.vector.tensor_tensor(
                                pbf[:, 0:128], pbf[:, 0:128], umask_b, OP.mult)
                        return pbf

                    for h in range(H):
                        av = psAv.tile([65, 512], F32, tag="Av", name=f"av{h}")
                        avs[h] = av
                        pbfs = {}

                        def emit_av(idx):
                            kt, qlo, qhi = KTQ[idx]
                            nc.tensor.matmul(
                                av[:, qlo:qhi], v65[kt][:, 65 * h:65 * h + 65],
                                pbfs[kt][:, 0:qhi - qlo],
                                start=(idx == 0), stop=(idx == 7))

                        # 2-deep software pipeline: sc runs ahead of av so
                        # the PE never waits on the ACT exp.
                        for idx, (kt, qlo, qhi) in enumerate(KTQ):
                            pbfs[kt] = emit_sc(h, kt, qlo, qhi)
                            if idx >= 2:
                                emit_av(idx - 2)
                        emit_av(6)
                        emit_av(7)
                        with nc.allow_low_precision(reason="softmax denom"):
                            nc.vector.reciprocal(rdena[:, h, :],
                                                 av[64:65, :])
                        if h > 0:
                            head_epilogue(h - 1)
                    head_epilogue(H - 1)

                    # output projection + store
                    ofin = cps.tile([128, DT, 512], F32, tag="ofin", name="ofin")
                    for do in range(DT):
                        po = psC.tile([128, 512], F32, tag="Cmm", name=f"o_ps{do}")
                        for ki in range(DT):
                            nc.tensor.matmul(po, swoT_b[ki][:, 128 * do:128 * (do + 1)],
                                             oTb[ki], start=(ki == 0),
                                             stop=(ki == DT - 1))
                        nc.scalar.copy(ofin[:, do, :], po)
                    nc.sync.dma_start(
                        out=out_d.rearrange("(a p) d -> p a d", p=128),
                        in_=ofin)

            for _bi in range(nbody):
                one_body(_bi)
    return nc


_CACHE = {}


def _get_nc(nbody=1):
    key = f"nc{nbody}"
    if key not in _CACHE:
        nc = build(nbody)
        split_waits(nc)
        _CACHE[key] = nc
    return _CACHE[key]


def prepare_in_maps(x, meta_memory, lmm_w, w_q, w_k, w_v, w_lr,
                    swa_wq, swa_wk, swa_wv, swa_wo):
    import ml_dtypes
    bfd = ml_dtypes.bfloat16

    x = np.asarray(x, np.float32)
    meta_memory = np.asarray(meta_memory, np.float32)
    lmm_w = np.asarray(lmm_w, np.float32)
    xm = np.concatenate(
        [np.broadcast_to(meta_memory, (B,) + meta_memory.shape), x], axis=1)
    xmb = xm.astype(bfd)
    xfb = xmb.reshape(NTOK, D)

    tri = np.arange(128)
    lmask_np = (tri[None, :] < tri[:, None]).astype(bfd)   # qj < ki
    umask_np = (tri[None, :] >= tri[:, None]).astype(bfd)  # qj >= ki
    ident_np = np.eye(128, dtype=np.float32)

    common = {
        "lmask": lmask_np, "umask": umask_np,
        "identb": ident_np.astype(bfd),
        "wkT": np.ascontiguousarray(np.asarray(w_k, np.float32).T).astype(bfd),
        "wvT": np.ascontiguousarray(np.asarray(w_v, np.float32).T).astype(bfd),
        "wlrT": np.ascontiguousarray(np.asarray(w_lr, np.float32).T).astype(bfd),
        "w0T": np.ascontiguousarray(lmm_w[0].T).astype(bfd),
        "w1T": np.ascontiguousarray(lmm_w[1].T).astype(bfd),
        "w1n": np.ascontiguousarray(lmm_w[1]).astype(bfd),
        "wqT": np.ascontiguousarray(np.asarray(w_q, np.float32).T).astype(bfd),
        "swqT": np.ascontiguousarray(np.asarray(swa_wq, np.float32).T).astype(bfd),
        "swkT": np.ascontiguousarray(np.asarray(swa_wk, np.float32).T).astype(bfd),
        "swvT": np.ascontiguousarray(np.asarray(swa_wv, np.float32).T).astype(bfd),
        "swoT": np.ascontiguousarray(np.asarray(swa_wo, np.float32).T).astype(bfd),
    }
    in_maps = []
    for c in range(NCORES):
        xa = np.zeros((D, TAP), bfd)
        xa[:, :TA] = xfb[TA * c:TA * (c + 1)].T
        b, r = c // 4, c % 4
        t1 = M + 512 * (r + 1)
        lo = max(t1 - TC, 0)
        pad = TC - (t1 - lo)
        xcm = np.zeros((D, TC), bfd)
        xcm[:, pad:] = xmb[b, lo:t1].T
        vk = np.zeros(TC, np.float32)
        vk[pad:] = 1.0
        mcore = dict(common)
        mcore["xaT"] = xa
        mcore["xcT"] = xcm
        mcore["vald01"] = np.ascontiguousarray(vk.reshape(8, 128).T)
        in_maps.append(mcore)
    return in_maps


def run_on_device(in_maps, nbody=1):
    nc = _get_nc(nbody)
    return bass_utils.run_bass_kernel_spmd(nc, in_maps,
                                           core_ids=list(range(NCORES)))


def kernel(**inputs):
    in_maps = prepare_in_maps(**inputs)
    res = run_on_device(in_maps)
    out = np.empty((B, S, D), np.float32)
    for c in range(NCORES):
        b, r = c // 4, c % 4
        out[b, 512 * r:512 * (r + 1), :] = res.results[c]["out"].T
    return out


# revision 3
# speedup vs baseline: 8438.3756x; 1.0663x over previous
"""NeuralMemory (scatter_memory) Trainium2 Bass kernel, 8-core SPMD. v2

Strategy (per core):
  Phase A (data-parallel over all B*T tokens, 528/core, zero-padded to 640):
    bf16 feature-major projections k/v/alr, 2-layer ResLinear forward +
    manual backward, PE-transpose the four dW operands into token-major
    layout (batched 4-wide PSUM evacuations), per-core partial dW^T into
    one fused [128, 4096] bf16 buffer.
  AllReduce the partial dW^T (bf16); the first AdamW step from zero state
    reduces to w_new = w*(1-lr*wd) - lr*sign(g).  Phase C weight/x loads
    and the q-projection overlap the collective.
  Phase C (each core owns one (batch, 512-token output range)): retrieval
    over own tokens + 512-token halo (padded to 1024) in f32r, sliding-
    window attention in bf16; key validity folded into the v-ones column;
    softmax denominators broadcast with K=1 PE matmuls (no DRAM round
    trips); output projection, write own slice.
"""
import numpy as np
import concourse.bass as bass
import concourse.tile as tile
import concourse.mybir as mybir
from concourse import bass_utils
import bass_rust

F32 = mybir.dt.float32
BF16 = mybir.dt.bfloat16
F32R = mybir.dt.float32r
AF = mybir.ActivationFunctionType
OP = mybir.AluOpType

NCORES = 8
B, S, D = 2, 2048, 512
M, C, H, WIN = 64, 16, 8, 512
N_LAYERS = 2
MAX_ALR = 0.01
LR, WD, EPS = 1e-3, 1e-2, 1e-8
T = M + S                  # 2112
NTOK = B * T               # 4224
TA = NTOK // NCORES        # 528 tokens/core in phase A
TAP = 640                  # padded phase-A width (5 x 128)
TC = 1024                  # phase-C halo+own width (8 x 128)
DT = D // 128              # 4 feature tiles
TT = TAP // 128            # 5 token tiles in phase A
HD = D // H                # 64
HALVES = ((0, 512), (512, TA - 512))   # 528 real tokens: 512 + 16


def split_waits(nc):
    """This walrus build encodes at most ONE sync wait per instruction.
    Hoist excess waits onto injected EventSemaphore instructions."""
    n = 0
    for fn in nc.m.functions:
        for blk in fn.blocks:
            newl = []
            for ins in blk.instructions:
                si = ins.sync_info
                if si is not None and len(si.on_wait) > 1:
                    waits = list(si.on_wait)
                    for w in waits[:-1]:
                        ev = mybir.InstEventSemaphore(
                            name=f"{ins.name}_w{n}", ins=[], outs=[])
                        ev.engine = ins.engine
                        ev.sync_info = bass_rust.SyncInfo(on_wait=[w], on_update=[])
                        newl.append(ev)
                        n += 1
                    ins.sync_info = bass_rust.SyncInfo(
                        on_wait=[waits[-1]], on_update=list(si.on_update))
                newl.append(ins)
            blk.instructions[:] = newl
    return n


_UID = [0]


def blocks(pool, nblk, width, dtype, tag):
    _UID[0] += 1
    t = pool.tile([128, nblk, width], dtype, tag=tag, name=f"{tag}_u{_UID[0]}")
    return [t[:, i, :] for i in range(nblk)]


def build(nbody=1, sim=False):
    nc = bass.Bass("TRN2", target_bir_lowering=False, debug=False,
                   num_devices=1 if sim else NCORES)

    # ---- DRAM I/O (bf16 throughout except the f32 output) ----
    xaT = nc.dram_tensor("xaT", [D, TAP], BF16, kind="ExternalInput").ap()
    xcT = nc.dram_tensor("xcT", [D, TC], BF16, kind="ExternalInput").ap()
    vald01 = nc.dram_tensor("vald01", [128, 8], F32, kind="ExternalInput").ap()
    lmask = nc.dram_tensor("lmask", [128, 128], BF16, kind="ExternalInput").ap()
    umask = nc.dram_tensor("umask", [128, 128], BF16, kind="ExternalInput").ap()
    identb = nc.dram_tensor("identb", [128, 128], BF16, kind="ExternalInput").ap()
    wkT = nc.dram_tensor("wkT", [D, D], BF16, kind="ExternalInput").ap()
    wvT = nc.dram_tensor("wvT", [D, D], BF16, kind="ExternalInput").ap()
    wlrT = nc.dram_tensor("wlrT", [D, 1], BF16, kind="ExternalInput").ap()
    w0T = nc.dram_tensor("w0T", [D, D], BF16, kind="ExternalInput").ap()
    w1T = nc.dram_tensor("w1T", [D, D], BF16, kind="ExternalInput").ap()
    w1n = nc.dram_tensor("w1n", [D, D], BF16, kind="ExternalInput").ap()
    wqT = nc.dram_tensor("wqT", [D, D], BF16, kind="ExternalInput").ap()
    swqT = nc.dram_tensor("swqT", [D, D], BF16, kind="ExternalInput").ap()
    swkT = nc.dram_tensor("swkT", [D, D], BF16, kind="ExternalInput").ap()
    swvT = nc.dram_tensor("swvT", [D, D], BF16, kind="ExternalInput").ap()
    swoT = nc.dram_tensor("swoT", [D, D], BF16, kind="ExternalInput").ap()
    out_d = nc.dram_tensor("out", [D, 512], F32, kind="ExternalOutput").ap()

    GW = N_LAYERS * DT * D     # 4096: fused gradient width

    with tile.TileContext(nc) as tc:
        with (
            tc.tile_pool(name="wpool", bufs=1) as wp,      # persistent
            tc.tile_pool(name="dramp", bufs=1, space="DRAM") as dramp,
        ):
            def load_w(src, name, eng=None):
                _UID[0] += 1
                t = wp.tile([128, DT, D], BF16, tag=name,
                            name=f"{name}_u{_UID[0]}")
                (eng or nc.sync).dma_start(
                    out=t, in_=src.rearrange("(a p) d -> p a d", p=128))
                return t, [t[:, i, :] for i in range(DT)]

            def one_body(body_i):
                # ---- bulk loads ----
                # phase A on the SP queue, most-urgent first
                xa_t = wp.tile([128, DT, TAP], BF16, tag="xa",
                               name=f"xa{body_i}")
                nc.sync.dma_start(
                    out=xa_t, in_=xaT.rearrange("(a p) d -> p a d", p=128))
                xa = [xa_t[:, i, :] for i in range(DT)]
                wlrT_r = wp.tile([128, DT, 1], BF16, tag="wlrT_r",
                                 name=f"wlr{body_i}")
                nc.sync.dma_start(
                    out=wlrT_r, in_=wlrT.rearrange("(a p) d -> p a d", p=128))
                _, wkT_r = load_w(wkT, "wkT_r")
                _, wvT_r = load_w(wvT, "wvT_r")
                w0T_t, w0T_r = load_w(w0T, "w0T_r")
                w1T_t, w1T_r = load_w(w1T, "w1T_r")
                _, w1n_r = load_w(w1n, "w1n_r")
                ident_b = wp.tile([128, 128], BF16, tag="ident_b",
                                  name=f"identb{body_i}")
                nc.sync.dma_start(out=ident_b, in_=identb)
                # phase C loads via SWDGE (Pool queue), off the SP HWDGE path
                _, wqT_r = load_w(wqT, "wqT_r", nc.gpsimd)
                xc_t = wp.tile([128, DT, TC], BF16, tag="xc",
                               name=f"xc{body_i}")
                nc.gpsimd.dma_start(
                    out=xc_t, in_=xcT.rearrange("(a p) d -> p a d", p=128))
                xc = [xc_t[:, i, :] for i in range(DT)]
                _, swqT_r = load_w(swqT, "swqT_r", nc.gpsimd)
                _, swkT_r = load_w(swkT, "swkT_r", nc.gpsimd)
                _, swvT_r = load_w(swvT, "swvT_r", nc.gpsimd)
                _, swoT_b = load_w(swoT, "swoT_b", nc.gpsimd)
                lmask_b = wp.tile([128, 128], BF16, tag="lmask_b",
                                  name=f"lm{body_i}")
                nc.gpsimd.dma_start(out=lmask_b, in_=lmask)
                umask_b = wp.tile([128, 128], BF16, tag="umask_b",
                                  name=f"um{body_i}")
                nc.gpsimd.dma_start(out=umask_b, in_=umask)
                vald_b = wp.tile([128, 8], F32, tag="vald_b",
                                 name=f"vald{body_i}")
                nc.gpsimd.dma_start(out=vald_b, in_=vald01)

                # w_new^T holder: [128, l, ki, D] f32r; prefill c1 * W^T
                wnT_t = wp.tile([128, N_LAYERS, DT, D], F32R, tag="wnT",
                                name=f"wnT{body_i}")
                wnT = [[wnT_t[:, l, j, :] for j in range(DT)]
                       for l in range(N_LAYERS)]
                c1 = 1.0 - LR * WD
                for l, wsrc in enumerate((w0T_t, w1T_t)):
                    nc.gpsimd.tensor_scalar_mul(wnT_t[:, l, :, :], wsrc, c1)

                # ================= PHASE A =================
                with (
                    tc.tile_pool(name="apool", bufs=3) as ap,
                    tc.tile_pool(name="apers", bufs=1) as aps,
                    tc.tile_pool(name="psA", bufs=4, space="PSUM") as psA,
                    tc.tile_pool(name="psTr", bufs=2, space="PSUM") as psTr,
                    tc.tile_pool(name="psDw", bufs=2, space="PSUM") as psDw,
                ):
                    def mmT(wtiles, rhs_tiles, name, evac):
                        for do in range(DT):
                            pss = []
                            for hf, (off, w) in enumerate(HALVES):
                                ps = psA.tile([128, 512], F32, tag="Amm",
                                              name=f"{name}_ps{do}_{hf}")
                                pss.append(ps[:, 0:w])
                            for ki in range(DT):
                                for hf, (off, w) in enumerate(HALVES):
                                    nc.tensor.matmul(
                                        pss[hf],
                                        wtiles[ki][:, 128 * do:128 * (do + 1)],
                                        rhs_tiles[ki][:, off:off + w],
                                        start=(ki == 0), stop=(ki == DT - 1))
                            for hf, (off, w) in enumerate(HALVES):
                                evac(do, off, w, pss[hf])

                    # alr first (row [1,TAP] -> DRAM round trip -> [128,5])
                    srow = aps.tile([1, TAP], F32, tag="srow", name="srow")
                    pa = psA.tile([128, 512], F32, tag="Amm", name="alr_ps")
                    pa2 = psA.tile([128, 512], F32, tag="Amm", name="alr_ps2")
                    for hf, (off, w) in enumerate(HALVES):
                        pz = (pa, pa2)[hf]
                        for ki in range(DT):
                            nc.tensor.matmul(pz[0:1, 0:w], wlrT_r[:, ki, :],
                                             xa[ki][:, off:off + w],
                                             start=(ki == 0), stop=(ki == DT - 1))
                        nc.scalar.activation(srow[:, off:off + w],
                                             pz[0:1, 0:w], AF.Sigmoid)
                    nc.vector.tensor_scalar_mul(srow, srow, 2.0 * MAX_ALR / D)
                    sband = dramp.tile([1, TAP], F32, tag="sband", name="sband")
                    nc.sync.dma_start(out=sband, in_=srow)
                    s_td = aps.tile([128, TT], F32, tag="s_td", name="s_td")
                    nc.sync.dma_start(
                        out=s_td,
                        in_=sband.opt().rearrange("a (c p) -> (a p) c", p=128))

                    # k projection
                    kT = blocks(aps, DT, TAP, BF16, "kT")
                    mmT(wkT_r, xa, "kproj",
                        lambda do, off, w, ps: nc.scalar.copy(
                            kT[do][:, off:off + w], ps))

                    # v projection (vTn = -v) immediately after: fills the
                    # PE gap while the z0 stage waits on kT evacuations
                    vTn = blocks(aps, DT, TAP, BF16, "vTn")
                    mmT(wvT_r, xa, "vproj",
                        lambda do, off, w, ps: nc.scalar.activation(
                            vTn[do][:, off:off + w], ps, AF.Copy, scale=-1.0))

                    # z0; x1 = k + silu(z0); d0
                    x1T = blocks(aps, DT, TAP, BF16, "x1T")
                    d0T = blocks(aps, DT, TAP, BF16, "d0T")

                    def z0_evac(do, off, w, ps):
                        sil = ap.tile([128, 512], F32, tag="silA",
                                      name=f"sil0_{do}_{off}")
                        nc.scalar.activation(sil[:, 0:w], ps, AF.Silu)
                        nc.scalar.activation(d0T[do][:, off:off + w],
                                             ps, AF.Derivative_silu)
                        nc.vector.tensor_tensor(
                            x1T[do][:, off:off + w],
                            kT[do][:, off:off + w], sil[:, 0:w], OP.add)
                    mmT(w0T_r, kT, "z0", z0_evac)

                    # token-major transpose targets, interleaved with the
                    # matmul stages to keep the PE fed
                    k_td = blocks(aps, TT, D, BF16, "k_td")
                    x1_td = blocks(aps, TT, D, BF16, "x1_td")
                    sdz1_td = blocks(aps, TT, D, BF16, "sdz1_td")
                    sdz0_td = blocks(aps, TT, D, BF16, "sdz0_td")

                    def transpose_into(dst, src, scale_s, name, eng):
                        for tt in range(TT):
                            rows = (128 if tt < TT - 1
                                    else TA - 128 * (TT - 1))
                            pt = psTr.tile([128, DT, 128], BF16, tag="Atr",
                                           name=f"tr_{name}_{tt}")
                            for do in range(DT):
                                nc.tensor.transpose(
                                    pt[0:rows, do, :],
                                    src[do][:, 128 * tt:128 * tt + rows],
                                    ident_b)
                            if scale_s:
                                nc.vector.tensor_scalar(
                                    dst[tt][0:rows, :], pt[0:rows, :, :],
                                    s_td[0:rows, tt:tt + 1], None, OP.mult)
                            else:
                                eng(dst[tt][0:rows, :], pt[0:rows, :, :])

                    gsb = aps.tile([128, GW], BF16, tag="gsb", name="gsb")

                    def dw_partial(l, x_td, z_td):
                        for j in range(DT):
                            pdw = psDw.tile([128, D], F32, tag="Adw",
                                            name=f"dw_ps{l}_{j}")
                            for tt in range(TT):
                                rows = (128 if tt < TT - 1
                                        else TA - 128 * (TT - 1))
                                nc.tensor.matmul(
                                    pdw,
                                    x_td[tt][0:rows, 128 * j:128 * (j + 1)],
                                    z_td[tt][0:rows, :], start=(tt == 0),
                                    stop=(tt == TT - 1))
                            eng = (nc.scalar.copy if (l * DT + j) % 2 == 0
                                   else nc.vector.tensor_copy)
                            eng(gsb[:, (l * DT + j) * D:(l * DT + j + 1) * D],
                                pdw)

                    transpose_into(k_td, kT, False, "k", nc.scalar.copy)


                    # z1; dx2 = (x1+silu(z1)) - v; dz1 = dx2*d1
                    dz1T = blocks(aps, DT, TAP, BF16, "dz1T")
                    dx2T = blocks(aps, DT, TAP, BF16, "dx2T")

                    def z1_evac(do, off, w, ps):
                        sil = ap.tile([128, 512], F32, tag="silA",
                                      name=f"sil1_{do}_{off}")
                        nc.scalar.activation(sil[:, 0:w], ps, AF.Silu)
                        d1 = ap.tile([128, 512], F32, tag="d1A",
                                     name=f"d1_{do}_{off}")
                        nc.scalar.activation(d1[:, 0:w], ps, AF.Derivative_silu)
                        x2 = ap.tile([128, 512], F32, tag="x2A",
                                     name=f"x2_{do}_{off}")
                        nc.vector.tensor_tensor(x2[:, 0:w],
                                                x1T[do][:, off:off + w],
                                                sil[:, 0:w], OP.add)
                        nc.vector.tensor_tensor(dx2T[do][:, off:off + w],
                                                x2[:, 0:w],
                                                vTn[do][:, off:off + w], OP.add)
                        nc.gpsimd.tensor_tensor(dz1T[do][:, off:off + w],
                                                dx2T[do][:, off:off + w],
                                                d1[:, 0:w], OP.mult)
                    mmT(w1T_r, x1T, "z1", z1_evac)

                    transpose_into(x1_td, x1T, False, "x1",
                                   nc.vector.tensor_copy)

                    # u = (dz1 @ W1)^T; dx1 = dx2 + u; dz0 = dx1*d0
                    dz0T = blocks(aps, DT, TAP, BF16, "dz0T")

                    def u_evac(do, off, w, ps):
                        dx1 = ap.tile([128, 512], BF16, tag="dx1A",
                                      name=f"dx1_{do}_{off}")
                        nc.vector.tensor_tensor(dx1[:, 0:w],
                                                dx2T[do][:, off:off + w],
                                                ps, OP.add)
                        nc.vector.tensor_tensor(dz0T[do][:, off:off + w],
                                                dx1[:, 0:w],
                                                d0T[do][:, off:off + w], OP.mult)
                    mmT(w1n_r, dz1T, "u", u_evac)

                    # layer-1 partials first: their operands finish earliest
                    transpose_into(sdz1_td, dz1T, True, "dz1", None)
                    dw_partial(1, x1_td, sdz1_td)
                    transpose_into(sdz0_td, dz0T, True, "dz0", None)
                    dw_partial(0, k_td, sdz0_td)

                    g_dram = dramp.tile([128, GW], BF16, tag="g_dram",
                                        name="g_dram")
                    gs_dram = dramp.tile([128, GW], BF16, tag="gs_dram",
                                         name="gs_dram")
                    nc.sync.dma_start(out=g_dram, in_=gsb)

                    if sim:
                        nc.gpsimd.dma_start(out=gs_dram, in_=g_dram)
                    else:
                        nc.gpsimd.collective_compute(
                            "AllReduce", OP.add,
                            replica_groups=[list(range(NCORES))],
                            ins=[g_dram.opt()], outs=[gs_dram.opt()])

                # ================= PHASE C =================
                with (
                    tc.tile_pool(name="cpool", bufs=2) as cp,
                    tc.tile_pool(name="cpb", bufs=4) as cpb,
                    tc.tile_pool(name="cpers", bufs=1) as cps,
                    tc.tile_pool(name="psC", bufs=3, space="PSUM") as psC,
                    tc.tile_pool(name="psS", bufs=2, space="PSUM") as psS,
                    tc.tile_pool(name="psAv", bufs=3, space="PSUM") as psAv,
                ):
                    def mmC(wtiles, rhs_tiles, name, out_cb, width=TC, roff=0,
                            offs=None):
                        if offs is None:
                            offs = range(0, width, 512)
                        for off in offs:
                            for do in range(DT):
                                ps = psC.tile([128, 512], F32, tag="Cmm",
                                              name=f"{name}_ps{do}_{off}")
                                for ki in range(DT):
                                    nc.tensor.matmul(
                                        ps, wtiles[ki][:, 128 * do:128 * (do + 1)],
                                        rhs_tiles[ki][:, roff + off:roff + off + 512],
                                        start=(ki == 0), stop=(ki == DT - 1))
                                out_cb(do, off, ps)

                    # queries: overlap the AllReduce (no dependency on wnT)
                    qT = blocks(cps, DT, TC, F32R, "qT")
                    mmC(wqT_r, xc, "q",
                        lambda do, off, ps: nc.scalar.copy(qT[do][:, off:off + 512], ps))

                    # AllReduce result -> sign -> w_new, chunked per layer/ki
                    # so the l0 matmuls start after the first chunk lands
                    gsum = cps.tile([128, GW], BF16, tag="gsum", name="gsum")
                    for hh in range(2):
                        sl = slice(hh * GW // 2, (hh + 1) * GW // 2)
                        nc.sync.dma_start(out=gsum[:, sl], in_=gs_dram[:, sl])
                    sgn = cps.tile([128, GW], BF16, tag="sgn", name="sgn")
                    for hh in range(2):
                        for ki in range(DT):
                            sl = slice((hh * DT + ki) * D,
                                       (hh * DT + ki + 1) * D)
                            nc.scalar.activation(sgn[:, sl], gsum[:, sl],
                                                 AF.Sign)
                            nc.vector.scalar_tensor_tensor(
                                wnT_t[:, hh, ki, :], sgn[:, sl], -LR,
                                wnT_t[:, hh, ki, :], OP.mult, OP.add)

                    r0T = blocks(cps, DT, TC, F32R, "r0T")

                    def l0_out(do, off, ps):
                        sil = cp.tile([128, 512], F32, tag="silC", name=f"l0s{do}_{off}")
                        nc.scalar.activation(sil, ps, AF.Silu)
                        nc.vector.tensor_tensor(r0T[do][:, off:off + 512],
                                                qT[do][:, off:off + 512], sil, OP.add)
                    mmC(wnT[0], qT, "l0", l0_out)

                    rT = blocks(cps, DT, TC, BF16, "rT")

                    def l1_out(do, off, ps):
                        sil = cp.tile([128, 512], F32, tag="silC", name=f"l1s{do}_{off}")
                        nc.scalar.activation(sil, ps, AF.Silu)
                        nc.vector.tensor_tensor(rT[do][:, off:off + 512],
                                                r0T[do][:, off:off + 512], sil, OP.add)
                    # own-token half (512:) first: it feeds sq, which gates
                    # every attention score matmul
                    mmC(wnT[1], r0T, "l1", l1_out, offs=(512, 0))

                    qTb = blocks(cps, DT, 512, BF16, "qTb")
                    mmC(swqT_r, rT, "sq",
                        lambda do, off, ps: nc.vector.tensor_copy(qTb[do], ps),
                        width=512, roff=512)

                    kTb = blocks(cps, DT, TC, BF16, "kTb")
                    v65 = blocks(cps, 8, H * 65, BF16, "v65")
                    ones8 = cps.tile([128, H, 1], BF16, tag="ones8", name="ones8")
                    nc.vector.memset(ones8, 1.0)

                    def v65_for(kt):
                        pv = psC.tile([128, 512], F32, tag="Cmm", name=f"v_ps{kt}")
                        for ki in range(DT):
                            nc.tensor.matmul(pv, rT[ki][:, 128 * kt:128 * (kt + 1)],
                                             swvT_r[ki], start=(ki == 0),
                                             stop=(ki == DT - 1))
                        v3 = v65[kt].rearrange("p (h c) -> p h c", c=65)
                        nc.vector.tensor_copy(v3[:, :, 0:64],
                                              pv.rearrange("p (h c) -> p h c", c=64))
                        nc.vector.tensor_scalar(v3[:, :, 64:65], ones8,
                                                vald_b[:, kt:kt + 1], None,
                                                OP.mult)

                    mmC(swkT_r, rT, "sk",
                        lambda do, off, ps: nc.vector.tensor_copy(
                            kTb[do][:, off:off + 512], ps),
                        offs=(512,))
                    for kt in range(4, 8):
                        v65_for(kt)
                    mmC(swkT_r, rT, "sk",
                        lambda do, off, ps: nc.vector.tensor_copy(
                            kTb[do][:, off:off + 512], ps),
                        offs=(0,))
                    for kt in range(4):
                        v65_for(kt)

                    # attention per head; denominators via K=1 PE broadcast.
                    # Head epilogues are software-pipelined one head behind so
                    # the PE never waits on the DVE reciprocal.
                    oTb = blocks(cps, DT, 512, BF16, "oTb")
                    rdena = cps.tile([1, H, 512], BF16, tag="rdena",
                                     name="rdena")
                    avs = [None] * H

                    onesb = cps.tile([1, 64], BF16, tag="onesb", name="onesb")
                    nc.vector.memset(onesb, 1.0)

                    def head_epilogue(h):
                        th, base = h // 2, 64 * (h % 2)
                        rbc = psC.tile([128, 512], F32, tag="Cmm",
                                       name=f"rbc{h}")
                        nc.tensor.matmul(rbc[0:64, :], onesb, rdena[:, h, :],
                                         start=True, stop=True)
                        rbs = cp.tile([64, 512], F32R, tag="rbs",
                                      name=f"rbs{h}")
                        nc.scalar.copy(rbs, rbc[0:64, :])
                        nc.vector.tensor_tensor(oTb[th][base:base + 64, :],
                                                avs[h][0:64, :], rbs,
                                                OP.mult)

                    # own-token key tiles (kt 4-7) are ready first, so they
                    # lead the PSUM accumulation order
                    KTQ = [(kt, 128 * max(0, kt - 4), min(512, 128 * (kt + 1)))
                           for kt in (4, 5, 6, 7, 0, 1, 2, 3)]

                    def emit_sc(h, kt, qlo, qhi):
                        # scores + exp + window masks -> pbf; returns pbf
                        th, base = h // 2, 64 * (h % 2)
                        wdt = qhi - qlo
                        sc = psS.tile([128, 512], F32, tag="Sc",
                                      name=f"sc{h}_{kt}")
                        nc.tensor.matmul(
                            sc[:, 0:wdt],
                            kTb[th][base:base + 64, 128 * kt:128 * (kt + 1)],
                            qTb[th][base:base + 64, qlo:qhi],
                            start=True, stop=True, tile_position=(base, 0))
                        pbf = cpb.tile([128, 512], BF16, tag="Pbf",
                                       name=f"p{h}_{kt}")
                        nc.scalar.activation(pbf[:, 0:wdt], sc[:, 0:wdt],
                                             AF.Exp, scale=0.125)
                        if kt <= 3:
                            nc.vector.tensor_tensor(
                                pbf[:, wdt - 128:wdt], pbf[:, wdt - 128:wdt],
                                lmask_b, OP.mult)
                        if kt >= 4:
                            nc.vector.tensor_tensor(
                                pbf[:, 0:128], pbf[:, 0:128], umask_b, OP.mult)
                        return pbf

                    for h in range(H):
                        av = psAv.tile([65, 512], F32, tag="Av", name=f"av{h}")
                        avs[h] = av
                        pbfs = {}

                        def emit_av(idx):
                            kt, qlo, qhi = KTQ[idx]
                            nc.tensor.matmul(
                                av[:, qlo:qhi], v65[kt][:, 65 * h:65 * h + 65],
                                pbfs[kt][:, 0:qhi - qlo],
                                start=(idx == 0), stop=(idx == 7))

                        # 2-deep software pipeline: sc runs ahead of av so
                        # the PE never waits on the ACT exp.
                        for idx, (kt, qlo, qhi) in enumerate(KTQ):
                            pbfs[kt] = emit_sc(h, kt, qlo, qhi)
                            if idx >= 2:
                                emit_av(idx - 2)
                        emit_av(6)
                        emit_av(7)
                        with nc.allow_low_precision(reason="softmax denom"):
                            nc.vector.reciprocal(rdena[:, h, :],
                                                 av[64:65, :])
                        if h > 0:
                            head_epilogue(h - 1)
                    head_epilogue(H - 1)

                    # output projection + store
                    ofin = cps.tile([128, DT, 512], F32, tag="ofin", name="ofin")
                    for do in range(DT):
                        po = psC.tile([128, 512], F32, tag="Cmm", name=f"o_ps{do}")
                        for ki in range(DT):
                            nc.tensor.matmul(po, swoT_b[ki][:, 128 * do:128 * (do + 1)],
                                             oTb[ki], start=(ki == 0),
                                             stop=(ki == DT - 1))
                        nc.scalar.copy(ofin[:, do, :], po)
                    nc.sync.dma_start(
                        out=out_d.rearrange("(a p) d -> p a d", p=128),
                        in_=ofin)

            for _bi in range(nbody):
                one_body(_bi)
    return nc


_CACHE = {}


def _get_nc(nbody=1):
    key = f"nc{nbody}"
    if key not in _CACHE:
        nc = build(nbody)
        split_waits(nc)
        _CACHE[key] = nc
    return _CACHE[key]


def prepare_in_maps(x, meta_memory, lmm_w, w_q, w_k, w_v, w_lr,
                    swa_wq, swa_wk, swa_wv, swa_wo):
    import ml_dtypes
    bfd = ml_dtypes.bfloat16

    x = np.asarray(x, np.float32)
    meta_memory = np.asarray(meta_memory, np.float32)
    lmm_w = np.asarray(lmm_w, np.float32)
    xm = np.concatenate(
        [np.broadcast_to(meta_memory, (B,) + meta_memory.shape), x], axis=1)
    xmb = xm.astype(bfd)
    xfb = xmb.reshape(NTOK, D)

    tri = np.arange(128)
    lmask_np = (tri[None, :] < tri[:, None]).astype(bfd)   # qj < ki
    umask_np = (tri[None, :] >= tri[:, None]).astype(bfd)  # qj >= ki
    ident_np = np.eye(128, dtype=np.float32)

    common = {
        "lmask": lmask_np, "umask": umask_np,
        "identb": ident_np.astype(bfd),
        "wkT": np.ascontiguousarray(np.asarray(w_k, np.float32).T).astype(bfd),
        "wvT": np.ascontiguousarray(np.asarray(w_v, np.float32).T).astype(bfd),
        "wlrT": np.ascontiguousarray(np.asarray(w_lr, np.float32).T).astype(bfd),
        "w0T": np.ascontiguousarray(lmm_w[0].T).astype(bfd),
        "w1T": np.ascontiguousarray(lmm_w[1].T).astype(bfd),
        "w1n": np.ascontiguousarray(lmm_w[1]).astype(bfd),
        "wqT": np.ascontiguousarray(np.asarray(w_q, np.float32).T).astype(bfd),
        "swqT": np.ascontiguousarray(np.asarray(swa_wq, np.float32).T).astype(bfd),
        "swkT": np.ascontiguousarray(np.asarray(swa_wk, np.float32).T).astype(bfd),
        "swvT": np.ascontiguousarray(np.asarray(swa_wv, np.float32).T).astype(bfd),
        "swoT": np.ascontiguousarray(np.asarray(swa_wo, np.float32).T).astype(bfd),
    }
    in_maps = []
    for c in range(NCORES):
        xa = np.zeros((D, TAP), bfd)
        xa[:, :TA] = xfb[TA * c:TA * (c + 1)].T
        b, r = c // 4, c % 4
        t1 = M + 512 * (r + 1)
        lo = max(t1 - TC, 0)
        pad = TC - (t1 - lo)
        xcm = np.zeros((D, TC), bfd)
        xcm[:, pad:] = xmb[b, lo:t1].T
        vk = np.zeros(TC, np.float32)
        vk[pad:] = 1.0
        mcore = dict(common)
        mcore["xaT"] = xa
        mcore["xcT"] = xcm
        mcore["vald01"] = np.ascontiguousarray(vk.reshape(8, 128).T)
        in_maps.append(mcore)
    return in_maps


def run_on_device(in_maps, nbody=1):
    nc = _get_nc(nbody)
    return bass_utils.run_bass_kernel_spmd(nc, in_maps,
                                           core_ids=list(range(NCORES)))


def kernel(**inputs):
    in_maps = prepare_in_maps(**inputs)
    res = run_on_device(in_maps)
    out = np.empty((B, S, D), np.float32)
    for c in range(NCORES):
        b, r = c // 4, c % 4
        out[b, 512 * r:512 * (r + 1), :] = res.results[c]["out"].T
    return out
